# revision 6
# baseline (speedup 1.0000x reference)
"""Luong attention kernel for Trainium2, data-parallel over batch on 8 cores.

Per core (batch b): query [64,256], keys [8192,256], Wa_w [256,256], Wa_b [256]
  qt   = query @ Wa_w        (transform the query instead of the keys)
  qb   = query @ Wa_b
  s    = qt @ keys^T + qb    == query @ (keys @ Wa_w^T + Wa_b)^T
  w    = softmax_k(s)                          -> weights out [64,8192]
  ctx  = (exp(s-m) @ keys) * 1/sum             -> context out [64,256]
"""
import numpy as np
from contextlib import ExitStack

import concourse.bass as bass
import bass_rust as _bass_rust
import concourse.mybir as mybir
import concourse.tile as tile
from concourse.bass_utils import run_bass_kernel_spmd
from concourse.masks import make_identity

F32 = mybir.dt.float32
AX = mybir.AxisListType
ACT = mybir.ActivationFunctionType

B, TQ, TK, H = 8, 64, 8192, 256
P = 128
NT = TK // P          # 64 key tiles of 128 rows
CH = 256              # scores chunk (one psum bank)
NCH = TK // CH        # 16 chunks
TPC = CH // P         # 4 key tiles per chunk


def build_nc():
    nc = bass.Bass()
    q_d = nc.declare_dram_parameter("query", [TQ, H], F32, isOutput=False)
    k_d = nc.declare_dram_parameter("keys", [TK, H], F32, isOutput=False)
    w_d = nc.declare_dram_parameter("Wa_w", [H, H], F32, isOutput=False)
    b_d = nc.declare_dram_parameter("Wa_b", [H], F32, isOutput=False)
    ctx_d = nc.declare_dram_parameter("context", [TQ, H], F32, isOutput=True)
    wts_d = nc.declare_dram_parameter("weights", [TQ, TK], F32, isOutput=True)

    with tile.TileContext(nc) as tc, ExitStack() as ex:
        const = ex.enter_context(tc.tile_pool(name="const", bufs=1))
        big = ex.enter_context(tc.tile_pool(name="big", bufs=1))
        ktp = ex.enter_context(tc.tile_pool(name="ktp", bufs=3))
        wtp = ex.enter_context(tc.tile_pool(name="wtp", bufs=3))
        ps_s = ex.enter_context(tc.tile_pool(name="ps_s", bufs=2, space="PSUM"))
        ps_kt = ex.enter_context(tc.tile_pool(name="ps_kt", bufs=2, space="PSUM"))
        ps_wt = ex.enter_context(tc.tile_pool(name="ps_wt", bufs=2, space="PSUM"))
        ps_c = ex.enter_context(tc.tile_pool(name="ps_c", bufs=1, space="PSUM"))
        ps_su = ps_kt  # setup tiles reuse the kt slots (same tag)

        # ---- persistent buffers
        keys_sb = big.tile([P, NT, H], F32)          # 64KB/part
        scores_sb = big.tile([TQ, TK], F32)          # 32KB/part
        expw_sb = big.tile([TQ, TK], F32)            # exp(s - m), unnormalized
        wnorm_sb = big.tile([TQ, TK], F32)           # normalized weights out

        ident = const.tile([P, P], F32)
        make_identity(nc, ident)
        w_sb = const.tile([P, 2, H], F32)            # W[o,h] o-chunks
        nc.sync.dma_start(out=w_sb, in_=w_d.rearrange("(c p) h -> p c h", p=P))
        q_sb = const.tile([TQ, H], F32)
        nc.sync.dma_start(out=q_sb, in_=q_d[:, :])
        b_sb = const.tile([P, 2], F32)
        nc.sync.dma_start(out=b_sb, in_=b_d.rearrange("(c p) -> p c", p=P))

        # keys: 8 x 1MB loads
        k_re = k_d.rearrange("(g t p) h -> g p t h", g=8, p=P)
        for g in range(8):
            nc.sync.dma_start(out=keys_sb[:, 8 * g:8 * g + 8, :], in_=k_re[g])

        # ---- setup: qtT [h,q] and qb [q,1]
        qT_ps = ps_su.tile([P, 2, TQ], F32, tag="kt")
        for j in range(2):
            nc.tensor.transpose(qT_ps[:, j, :], q_sb[:, P * j:P * (j + 1)],
                                ident[:TQ, :TQ])
        qT_sb = const.tile([P, 2, TQ], F32)
        nc.vector.tensor_copy(qT_sb, qT_ps)

        qtT_ps = ps_su.tile([P, 2, TQ], F32, tag="kt")
        for hc in range(2):
            for j in range(2):
                nc.tensor.matmul(qtT_ps[:, hc, :],
                                 w_sb[:, j, P * hc:P * (hc + 1)],
                                 qT_sb[:, j, :],
                                 start=(j == 0), stop=(j == 1))
        qtT_sb = const.tile([P, 2, TQ], F32)
        nc.vector.tensor_copy(qtT_sb, qtT_ps)

        qb_ps = ps_su.tile([TQ, 1], F32, tag="kt")
        for j in range(2):
            nc.tensor.matmul(qb_ps, qT_sb[:, j, :], b_sb[:, j:j + 1],
                             start=(j == 0), stop=(j == 1))
        qb_sb = const.tile([TQ, 1], F32)
        nc.vector.tensor_copy(qb_sb, qb_ps)

        # ---- phase 1: scores chunks of 512 k
        for c in range(NCH):
            kt_ps = ps_kt.tile([P, 2, CH], F32, tag="kt")  # keysT chunk, 1 bank
            for u in range(TPC):
                t = TPC * c + u
                for hc in range(2):
                    nc.tensor.transpose(kt_ps[:, hc, P * u:P * (u + 1)],
                                        keys_sb[:, t, P * hc:P * (hc + 1)],
                                        ident)
            kt_sb = ktp.tile([P, 2, CH], F32, tag="kt")
            if c % 2 == 0:
                nc.vector.tensor_copy(kt_sb, kt_ps)
            else:
                nc.scalar.copy(kt_sb, kt_ps)

            s_ps = ps_s.tile([TQ, CH], F32, tag="s")
            for hc in range(2):
                nc.tensor.matmul(s_ps, qtT_sb[:, hc, :], kt_sb[:, hc, :],
                                 start=(hc == 0), stop=(hc == 1))
            # copy psum->sbuf, adding the per-row bias qb
            nc.scalar.add(scores_sb[:, CH * c:CH * (c + 1)], s_ps, add=qb_sb)

        # ---- phase 2: softmax pieces
        negmax = const.tile([TQ, 1], F32)
        nc.vector.reduce_max(out=negmax, in_=scores_sb, axis=AX.X, negate=True)
        sums = const.tile([TQ, 1], F32)
        nc.scalar.activation(out=expw_sb, in_=scores_sb, func=ACT.Exp,
                             bias=negmax, scale=1.0, accum_out=sums)
        rsum = const.tile([TQ, 1], F32)
        nc.vector.reciprocal(rsum, sums)

        # normalized weights out (split across DVE/ACT), overlap with phase 3
        half = TK // 2
        nc.vector.tensor_scalar_mul(wnorm_sb[:, :half], expw_sb[:, :half], rsum)
        nc.scalar.mul(wnorm_sb[:, half:], expw_sb[:, half:], mul=rsum)
        nc.sync.dma_start(out=wts_d[:, :half], in_=wnorm_sb[:, :half])
        nc.sync.dma_start(out=wts_d[:, half:], in_=wnorm_sb[:, half:])

        # ---- phase 3: context^T accumulation over key tiles
        ctxT_ps0 = ps_c.tile([P, TQ], F32)
        ctxT_ps1 = ps_c.tile([P, TQ], F32)
        for grp in range(NT // 4):
            wt_ps = ps_wt.tile([P, 4, TQ], F32, tag="wt")
            for u in range(4):
                t = 4 * grp + u
                nc.tensor.transpose(wt_ps[:, u, :],
                                    expw_sb[:, P * t:P * (t + 1)],
                                    ident[:TQ, :TQ])
            wt_sb = wtp.tile([P, 4, TQ], F32, tag="wt")
            if grp % 2 == 0:
                nc.vector.tensor_copy(wt_sb, wt_ps)
            else:
                nc.scalar.copy(wt_sb, wt_ps)
            for u in range(4):
                t = 4 * grp + u
                nc.tensor.matmul(ctxT_ps0, keys_sb[:, t, 0:P], wt_sb[:, u, :],
                                 start=(t == 0), stop=(t == NT - 1))
                nc.tensor.matmul(ctxT_ps1, keys_sb[:, t, P:H], wt_sb[:, u, :],
                                 start=(t == 0), stop=(t == NT - 1))

        ctxT_sb = const.tile([P, 2, TQ], F32)
        nc.vector.tensor_copy(ctxT_sb[:, 0, :], ctxT_ps0)
        nc.vector.tensor_copy(ctxT_sb[:, 1, :], ctxT_ps1)
        ctx_ps = ps_wt.tile([TQ, 2, P], F32, tag="wt")
        for hc in range(2):
            nc.tensor.transpose(ctx_ps[:, hc, :], ctxT_sb[:, hc, :], ident)
        ctx_sb = const.tile([TQ, H], F32)
        nc.vector.tensor_scalar_mul(ctx_sb.rearrange("q (c p) -> q c p", c=2),
                                    ctx_ps, rsum)
        nc.sync.dma_start(out=ctx_d[:, :], in_=ctx_sb)

    # split multi-sem waits: matmul waits -> ldweights, then event-sem
    # funnels so every instruction carries at most one HW wait slot
    _bass_rust.move_matmul_waits_to_ldweights(nc.m)
    _bass_rust.generate_event_semaphores(nc)
    return nc


_NC = None


def _get_nc():
    global _NC
    if _NC is None:
        _NC = build_nc()
    return _NC


def kernel(query, keys, Wa_w, Wa_b, _trace=False):
    nc = _get_nc()
    in_maps = [
        {"query": np.ascontiguousarray(query[i], dtype=np.float32),
         "keys": np.ascontiguousarray(keys[i], dtype=np.float32),
         "Wa_w": np.ascontiguousarray(Wa_w, dtype=np.float32),
         "Wa_b": np.ascontiguousarray(Wa_b, dtype=np.float32)}
        for i in range(B)
    ]
    res = run_bass_kernel_spmd(nc, in_maps, list(range(B)), trace=_trace)
    ctx = np.stack([res.results[i]["context"] for i in range(B)])
    wts = np.stack([res.results[i]["weights"] for i in range(B)])
    if _trace:
        return (ctx, wts), res
    return (ctx, wts)


# revision 9
# speedup vs baseline: 1.2267x; 1.2267x over previous
"""Luong attention kernel for Trainium2, data-parallel over batch on 8 cores.

Per core (batch b): query [64,256], keys [8192,256], Wa_w [256,256], Wa_b [256]
  qt   = query @ Wa_w        (transform the query instead of the keys)
  qb   = query @ Wa_b
  s    = qt @ keys^T + qb    == query @ (keys @ Wa_w^T + Wa_b)^T
  w    = softmax_k(s)                          -> weights out [64,8192]
  ctx  = (exp(s-m) @ keys) * 1/sum             -> context out [64,256]
"""
import numpy as np
from contextlib import ExitStack

import concourse.bass as bass
import bass_rust as _bass_rust
import concourse.mybir as mybir
import concourse.tile as tile
from concourse.bass_utils import run_bass_kernel_spmd
from concourse.masks import make_identity

F32 = mybir.dt.float32
AX = mybir.AxisListType
ACT = mybir.ActivationFunctionType

B, TQ, TK, H = 8, 64, 8192, 256
P = 128
NT = TK // P          # 64 key tiles of 128 rows
CH = 256              # scores chunk (one psum bank)
NCH = TK // CH        # 16 chunks
TPC = CH // P         # 4 key tiles per chunk


def build_nc():
    nc = bass.Bass()
    q_d = nc.declare_dram_parameter("query", [TQ, H], F32, isOutput=False)
    k_d = nc.declare_dram_parameter("keys", [TK, H], F32, isOutput=False)
    w_d = nc.declare_dram_parameter("Wa_w", [H, H], F32, isOutput=False)
    b_d = nc.declare_dram_parameter("Wa_b", [H], F32, isOutput=False)
    ctx_d = nc.declare_dram_parameter("context", [TQ, H], F32, isOutput=True)
    wts_d = nc.declare_dram_parameter("weights", [TQ, TK], F32, isOutput=True)

    with tile.TileContext(nc) as tc, ExitStack() as ex:
        const = ex.enter_context(tc.tile_pool(name="const", bufs=1))
        big = ex.enter_context(tc.tile_pool(name="big", bufs=1))
        ktp = ex.enter_context(tc.tile_pool(name="ktp", bufs=3))
        wtp = ex.enter_context(tc.tile_pool(name="wtp", bufs=3))
        ps_s = ex.enter_context(tc.tile_pool(name="ps_s", bufs=2, space="PSUM"))
        ps_kt = ex.enter_context(tc.tile_pool(name="ps_kt", bufs=2, space="PSUM"))
        ps_wt = ex.enter_context(tc.tile_pool(name="ps_wt", bufs=2, space="PSUM"))
        ps_c = ex.enter_context(tc.tile_pool(name="ps_c", bufs=1, space="PSUM"))
        ps_su = ps_kt  # setup tiles reuse the kt slots (same tag)

        # ---- persistent buffers
        keys_sb = big.tile([P, NT, H], F32)          # 64KB/part
        scores_sb = big.tile([TQ, TK], F32)          # 32KB/part
        expw_sb = big.tile([TQ, TK], F32)            # exp(s - m), unnormalized
        wnorm_sb = big.tile([TQ, TK], F32)           # normalized weights out

        ident = const.tile([P, P], F32)
        make_identity(nc, ident)
        w_sb = const.tile([P, 2, H], F32)            # W[o,h] o-chunks
        nc.sync.dma_start(out=w_sb, in_=w_d.rearrange("(c p) h -> p c h", p=P))
        q_sb = const.tile([TQ, H], F32)
        nc.sync.dma_start(out=q_sb, in_=q_d[:, :])
        b_sb = const.tile([P, 2], F32)
        nc.sync.dma_start(out=b_sb, in_=b_d.rearrange("(c p) -> p c", p=P))

        # keys: 8 x 1MB loads
        k_re = k_d.rearrange("(g t p) h -> g p t h", g=8, p=P)
        for g in range(8):
            nc.sync.dma_start(out=keys_sb[:, 8 * g:8 * g + 8, :], in_=k_re[g])

        # ---- setup: qtT [h,q] and qb [q,1]
        qT_ps = ps_su.tile([P, 2, TQ], F32, tag="kt")
        for j in range(2):
            nc.tensor.transpose(qT_ps[:, j, :], q_sb[:, P * j:P * (j + 1)],
                                ident[:TQ, :TQ])
        qT_sb = const.tile([P, 2, TQ], F32)
        nc.vector.tensor_copy(qT_sb, qT_ps)

        qtT_ps = ps_su.tile([P, 2, TQ], F32, tag="kt")
        for hc in range(2):
            for j in range(2):
                nc.tensor.matmul(qtT_ps[:, hc, :],
                                 w_sb[:, j, P * hc:P * (hc + 1)],
                                 qT_sb[:, j, :],
                                 start=(j == 0), stop=(j == 1))
        qtT_sb = const.tile([P, 2, TQ], F32)
        nc.vector.tensor_copy(qtT_sb, qtT_ps)

        qb_ps = ps_su.tile([TQ, 1], F32, tag="kt")
        for j in range(2):
            nc.tensor.matmul(qb_ps, qT_sb[:, j, :], b_sb[:, j:j + 1],
                             start=(j == 0), stop=(j == 1))
        qb_sb = const.tile([TQ, 1], F32)
        nc.vector.tensor_copy(qb_sb, qb_ps)

        # ---- phase 1: scores chunks of 512 k
        for c in range(NCH):
            kt_ps = ps_kt.tile([P, 2, CH], F32, tag="kt")  # keysT chunk, 1 bank
            for u in range(TPC):
                t = TPC * c + u
                for hc in range(2):
                    nc.tensor.transpose(kt_ps[:, hc, P * u:P * (u + 1)],
                                        keys_sb[:, t, P * hc:P * (hc + 1)],
                                        ident)
            kt_sb = ktp.tile([P, 2, CH], F32, tag="kt")
            if c % 2 == 0:
                nc.vector.tensor_copy(kt_sb, kt_ps)
            else:
                nc.scalar.copy(kt_sb, kt_ps)

            s_ps = ps_s.tile([TQ, CH], F32, tag="s")
            for hc in range(2):
                nc.tensor.matmul(s_ps, qtT_sb[:, hc, :], kt_sb[:, hc, :],
                                 start=(hc == 0), stop=(hc == 1))
            # copy psum->sbuf, adding the per-row bias qb
            nc.scalar.add(scores_sb[:, CH * c:CH * (c + 1)], s_ps, add=qb_sb)

        # ---- phase 2: softmax pieces
        negmax = const.tile([TQ, 1], F32)
        nc.vector.reduce_max(out=negmax, in_=scores_sb, axis=AX.X, negate=True)
        sums = const.tile([TQ, 1], F32)
        nc.scalar.activation(out=expw_sb, in_=scores_sb, func=ACT.Exp,
                             bias=negmax, scale=1.0, accum_out=sums)
        rsum = const.tile([TQ, 1], F32)
        nc.vector.reciprocal(rsum, sums)

        # normalized weights out (split across DVE/ACT), overlap with phase 3
        half = TK // 2
        nc.vector.tensor_scalar_mul(wnorm_sb[:, :half], expw_sb[:, :half], rsum)
        nc.scalar.mul(wnorm_sb[:, half:], expw_sb[:, half:], mul=rsum)
        nc.sync.dma_start(out=wts_d[:, :half], in_=wnorm_sb[:, :half])
        nc.sync.dma_start(out=wts_d[:, half:], in_=wnorm_sb[:, half:])

        # ---- phase 3: context^T accumulation over key tiles
        ctxT_ps0 = ps_c.tile([P, TQ], F32)
        ctxT_ps1 = ps_c.tile([P, TQ], F32)
        for grp in range(NT // 4):
            wt_ps = ps_wt.tile([P, 4, TQ], F32, tag="wt")
            for u in range(4):
                t = 4 * grp + u
                nc.tensor.transpose(wt_ps[:, u, :],
                                    expw_sb[:, P * t:P * (t + 1)],
                                    ident[:TQ, :TQ])
            wt_sb = wtp.tile([P, 4, TQ], F32, tag="wt")
            if grp % 2 == 0:
                nc.vector.tensor_copy(wt_sb, wt_ps)
            else:
                nc.scalar.copy(wt_sb, wt_ps)
            for u in range(4):
                t = 4 * grp + u
                nc.tensor.matmul(ctxT_ps0, keys_sb[:, t, 0:P], wt_sb[:, u, :],
                                 start=(t == 0), stop=(t == NT - 1))
                nc.tensor.matmul(ctxT_ps1, keys_sb[:, t, P:H], wt_sb[:, u, :],
                                 start=(t == 0), stop=(t == NT - 1))

        ctxT_sb = const.tile([P, 2, TQ], F32)
        nc.vector.tensor_copy(ctxT_sb[:, 0, :], ctxT_ps0)
        nc.vector.tensor_copy(ctxT_sb[:, 1, :], ctxT_ps1)
        ctx_ps = ps_wt.tile([TQ, 2, P], F32, tag="wt")
        for hc in range(2):
            nc.tensor.transpose(ctx_ps[:, hc, :], ctxT_sb[:, hc, :], ident)
        ctx_sb = const.tile([TQ, H], F32)
        nc.vector.tensor_scalar_mul(ctx_sb.rearrange("q (c p) -> q c p", c=2),
                                    ctx_ps, rsum)
        nc.sync.dma_start(out=ctx_d[:, :], in_=ctx_sb)

    # split multi-sem waits: matmul waits -> ldweights, then event-sem
    # funnels so every instruction carries at most one HW wait slot
    _bass_rust.move_matmul_waits_to_ldweights(nc.m)
    _bass_rust.generate_event_semaphores(nc)
    return nc


_RUNNER = None


def _make_runner():
    """Build the SPMD executable once; reuse across kernel() calls."""
    import jax
    from jax.sharding import Mesh, PartitionSpec
    from jax.experimental.shard_map import shard_map
    from concourse import bass2jax

    nc = build_nc()
    bass2jax.install_neuronx_cc_hook()

    in_names, out_names, out_avals, zero_outs = [], [], [], []
    for alloc in nc.m.functions[0].allocations:
        if not isinstance(alloc, mybir.MemoryLocationSet):
            continue
        name = alloc.memorylocations[0].name
        pname = nc.partition_id_tensor.name if nc.partition_id_tensor else None
        if alloc.kind == "ExternalInput":
            if name != pname:
                in_names.append(name)
        elif alloc.kind == "ExternalOutput":
            out_names.append(name)
            shape = tuple(alloc.tensor_shape)
            dtype = mybir.dt.np(alloc.dtype)
            out_avals.append(jax.core.ShapedArray(shape, dtype))
            zero_outs.append(np.zeros((B * shape[0],) + shape[1:], dtype))
    n_params = len(in_names)
    all_names = in_names + out_names
    if nc.partition_id_tensor:
        all_names = all_names + [nc.partition_id_tensor.name]

    def _body(*args):
        operands = list(args)
        if nc.partition_id_tensor:
            operands.append(bass2jax.partition_id_tensor())
        outs = bass2jax._bass_exec_p.bind(
            *operands,
            out_avals=tuple(out_avals),
            in_names=tuple(all_names),
            out_names=tuple(out_names),
            lowering_input_output_aliases=(),
            sim_require_finite=True,
            sim_require_nnan=True,
            nc=nc,
        )
        return tuple(outs)

    devices = jax.devices()[:B]
    mesh = Mesh(np.asarray(devices), ("core",))
    n_outs = len(out_names)
    sharded = jax.jit(
        shard_map(_body, mesh=mesh,
                  in_specs=(PartitionSpec("core"),) * (n_params + n_outs),
                  out_specs=(PartitionSpec("core"),) * n_outs,
                  check_rep=False),
        donate_argnums=tuple(range(n_params, n_params + n_outs)),
        keep_unused=True,
    )

    def run(in_map_global):
        args = [in_map_global[n] for n in in_names]
        zouts = [np.zeros_like(z) for z in zero_outs]
        outs = sharded(*args, *zouts)
        return {n: np.asarray(o) for n, o in zip(out_names, outs)}

    return run


def kernel(query, keys, Wa_w, Wa_b):
    global _RUNNER
    if _RUNNER is None:
        _RUNNER = _make_runner()
    f32 = np.float32
    # global arrays: per-core shapes concatenated along axis 0
    gin = {
        "query": np.ascontiguousarray(query, f32).reshape(B * TQ, H),
        "keys": np.ascontiguousarray(keys, f32).reshape(B * TK, H),
        "Wa_w": np.tile(np.ascontiguousarray(Wa_w, f32), (B, 1)),
        "Wa_b": np.tile(np.ascontiguousarray(Wa_b, f32), B),
    }
    out = _RUNNER(gin)
    ctx = out["context"].reshape(B, TQ, H)
    wts = out["weights"].reshape(B, TQ, TK)
    return (ctx, wts)
